# revision 7
# baseline (speedup 1.0000x reference)
"""Cross-attention (B=2, NQ=2048, NK=4096, D=1024, H=16) on 8 trn2 cores.

Sharding: heads across cores. Core c owns heads 2c, 2c+1 (d-slice
[128c, 128c+128)). Each core projects q/k/v for its heads (full q,k,v
replicated), runs masked-softmax attention for its 2 heads over both
batches, then an AllGather of the per-core attention outputs
(d-on-partitions layout) feeds a column-sharded output projection.

Layouts keep the contraction dim on SBUF partitions everywhere:
  qhT [128, B*NQ], khT [128, B*NK]   (2 heads stacked: rows 0-63 / 64-127)
  vh  [128, 64chunks, 130]           (k-rows on partitions; per chunk
                                      cols 0-63 h0-data, 64 h0-mask01,
                                      65-128 h1-data, 129 h1-mask01)

Masking is multiplicative via vh (masked k rows zeroed; the "ones"
column used for the softmax denominator holds the 0/1 mask), so the
exp activation needs no per-chunk bias and spans whole psum tiles.
Softmax skips max-subtraction (scores*SCALE ~ N(0,1); exp safe in fp32).
This matches the reference exactly: where(mask,-1e20)*SCALE -> exp = 0.

Matmuls run in float32r (full PE rate at N>=256). Walrus requires fp32r
matmul operands to be *produced* as fp32r, so every operand tensor
(dram params, sbuf tiles) is declared float32r and the producing
DVE/ACT ops write fp32r directly. The v-projection has N=128 (fp32r
runs 1/4 rate below N=256) so it runs in bf16 instead.
"""

import numpy as np
import ml_dtypes

B, NQ, NK, D, H = 2, 2048, 4096, 1024, 16
HD = D // H
SCALE = HD**-0.5
NCORES = 8
DC = 128  # d-slice per core (2 heads * 64)
NQF = B * NQ  # 4096 flat query rows
NKF = B * NK  # 8192 flat key rows
NKC = NKF // 128  # 64 k-chunks total
QB = 512  # q block size

_CACHE = {}


def _build_bass():
    import concourse.mybir as mybir
    import concourse.tile as tile
    from concourse import bacc

    fp32 = mybir.dt.float32
    fp32r = mybir.dt.float32r
    bf16 = mybir.dt.bfloat16
    EXP = mybir.ActivationFunctionType.Exp
    DCH = D // 128  # 8 d-chunks

    nc = bacc.Bacc(None, num_devices=NCORES)

    qT_d = nc.declare_dram_parameter("qT", [D, NQF], fp32r, isOutput=False)
    kT_d = nc.declare_dram_parameter("kT", [D, NKF], fp32r, isOutput=False)
    vT_d = nc.declare_dram_parameter("vT", [D, NKF], bf16, isOutput=False)
    wq_d = nc.declare_dram_parameter("wq", [D, DC], fp32r, isOutput=False)
    wk_d = nc.declare_dram_parameter("wk", [D, DC], fp32r, isOutput=False)
    wv_d = nc.declare_dram_parameter("wv", [D, DC], bf16, isOutput=False)
    wp_d = nc.declare_dram_parameter("wp", [D, DC], fp32r, isOutput=False)
    bp_d = nc.declare_dram_parameter("bp", [DC, 1], fp32, isOutput=False)
    m01_d = nc.declare_dram_parameter("m01", [128, NKC], fp32, isOutput=False)
    xout_d = nc.declare_dram_parameter("xout", [DC, NQF], fp32, isOutput=True)

    with tile.TileContext(nc) as tc:
        with (
            tc.tile_pool(name="wpool", bufs=1) as wpool,
            tc.tile_pool(name="resid", bufs=1) as resid,
            tc.tile_pool(name="iostream", bufs=4) as iostream,
            tc.tile_pool(name="vstream", bufs=2) as vstream,
            tc.tile_pool(name="expp", bufs=4) as expp,
            tc.tile_pool(name="recipp", bufs=2) as recipp,
            tc.tile_pool(name="xpool", bufs=3) as xpool,
            tc.tile_pool(name="psA", bufs=2, space="PSUM") as psA,
            tc.tile_pool(name="psS", bufs=3, space="PSUM") as psS,
            tc.tile_pool(name="dramp", bufs=1, space="DRAM") as dramp,
        ):
            # ---- resident weights ----
            wq_sb = wpool.tile([128, DCH, DC], fp32r, name="wq_sb")
            wk_sb = wpool.tile([128, DCH, DC], fp32r, name="wk_sb")
            wv_sb = wpool.tile([128, DCH, DC], bf16, name="wv_sb")
            wp_sb = wpool.tile([128, DCH, DC], fp32r, name="wp_sb")
            nc.sync.dma_start(wq_sb[:], wq_d.rearrange("(o p) m -> p o m", p=128))
            nc.sync.dma_start(wk_sb[:], wk_d.rearrange("(o p) m -> p o m", p=128))
            nc.sync.dma_start(wv_sb[:], wv_d.rearrange("(o p) m -> p o m", p=128))
            nc.sync.dma_start(wp_sb[:], wp_d.rearrange("(o p) m -> p o m", p=128))
            bp_sb = wpool.tile([DC, 1], fp32, name="bp_sb")
            nc.sync.dma_start(bp_sb[:], bp_d[:])
            m01_sb = wpool.tile([128, NKC], fp32, name="m01_sb")
            nc.sync.dma_start(m01_sb[:], m01_d[:])
            ones_f = wpool.tile([1, 64], fp32, name="ones_f")
            nc.vector.memset(ones_f[:], 1.0)
            ones_sb = wpool.tile([1, 64], fp32r, name="ones_sb")
            nc.vector.tensor_copy(ones_sb[:], ones_f[:])

            # ---- resident activations ----
            qhT = resid.tile([128, NQF], fp32r, name="qhT")
            khT = resid.tile([128, NKF], fp32r, name="khT")
            vh = resid.tile([128, NKC, 130], fp32r, name="vh")
            ocT = resid.tile([128, NQF], fp32r, name="ocT")

            # ---- P1a: q/k head projections (transposed out, d on parts) ----
            for dst, src_d, w_sb, nblk in (
                (qhT, qT_d, wq_sb, NQF // QB),
                (khT, kT_d, wk_sb, NKF // QB),
            ):
                for blk in range(nblk):
                    ps = psA.tile([128, QB], fp32, tag="psA", name="ps_proj")
                    for dc in range(DCH):
                        t_in = iostream.tile([128, QB], fp32r, tag="ios", name="t_in")
                        nc.sync.dma_start(
                            t_in[:],
                            src_d[dc * 128 : (dc + 1) * 128, blk * QB : (blk + 1) * QB],
                        )
                        nc.tensor.matmul(
                            ps[:],
                            w_sb[:, dc, :],
                            t_in[:],
                            start=(dc == 0),
                            stop=(dc == DCH - 1),
                        )
                    nc.vector.tensor_copy(dst[:, blk * QB : (blk + 1) * QB], ps[:])

            # ---- P1b: v projection, masked; mask01 as denominator columns --
            nc.vector.tensor_copy(vh[:, :, 64], m01_sb[:])
            nc.vector.tensor_copy(vh[:, :, 129], m01_sb[:])
            VB = 512  # k-rows per streamed v block (4 chunks)
            for vb in range(NKF // VB):
                vt = vstream.tile([128, DCH, VB], bf16, tag="vs", name="vt")
                nc.sync.dma_start(
                    vt[:],
                    vT_d.rearrange("(o p) n -> p o n", p=128)[
                        :, :, vb * VB : (vb + 1) * VB
                    ],
                )
                for ci in range(VB // 128):
                    g = vb * (VB // 128) + ci
                    psv = psA.tile([128, DC], fp32, tag="psA", name="psv")
                    for dc in range(DCH):
                        nc.tensor.matmul(
                            psv[:],
                            vt[:, dc, ci * 128 : (ci + 1) * 128],
                            wv_sb[:, dc, :],
                            start=(dc == 0),
                            stop=(dc == DCH - 1),
                        )
                    nc.vector.tensor_scalar_mul(
                        vh[:, g, 0:64], psv[:, 0:64], m01_sb[:, g : g + 1]
                    )
                    nc.vector.tensor_scalar_mul(
                        vh[:, g, 65:129], psv[:, 64:128], m01_sb[:, g : g + 1]
                    )

            # ---- P2: attention per (batch, q-block) ----
            KCB = NK // 128  # 32 k-chunks per batch
            for b in range(B):
                for qb in range(NQ // QB):
                    col0 = b * NQ + qb * QB
                    po0 = psA.tile([65, QB], fp32, tag="psA", name="po0")
                    po1 = psA.tile([65, QB], fp32, tag="psA", name="po1")
                    for kc in range(KCB):
                        g = b * KCB + kc
                        pss = psS.tile([128, 2 * QB], fp32, tag="st", name="pss")
                        # scores^T for both heads, row-packed (K=64 each)
                        nc.tensor.matmul(
                            pss[:, 0:QB],
                            khT[0:64, g * 128 : (g + 1) * 128],
                            qhT[0:64, col0 : col0 + QB],
                            start=True,
                            stop=True,
                            tile_position=(0, 0),
                        )
                        nc.tensor.matmul(
                            pss[:, QB : 2 * QB],
                            khT[64:128, g * 128 : (g + 1) * 128],
                            qhT[64:128, col0 : col0 + QB],
                            start=True,
                            stop=True,
                            tile_position=(64, 0),
                        )
                        et = expp.tile([128, 2 * QB], fp32r, tag="et", name="et")
                        nc.scalar.activation(et[:], pss[:], EXP, scale=SCALE)
                        nc.tensor.matmul(
                            po0[:],
                            vh[:, g, 0:65],
                            et[:, 0:QB],
                            start=(kc == 0),
                            stop=(kc == KCB - 1),
                        )
                        nc.tensor.matmul(
                            po1[:],
                            vh[:, g, 65:130],
                            et[:, QB : 2 * QB],
                            start=(kc == 0),
                            stop=(kc == KCB - 1),
                        )
                    # normalize: rows 0-63 data, row 64 denominator
                    for h, po in ((0, po0), (1, po1)):
                        rec = recipp.tile([1, QB], fp32r, tag="rec", name="rec")
                        with nc.allow_low_precision(
                            reason="fp32r recip feeds fp32r bcast matmul"
                        ):
                            nc.vector.reciprocal(rec[:], po[64:65, :])
                        bc = psS.tile([64, QB], fp32, tag="st", name="bc")
                        nc.tensor.matmul(
                            bc[:], ones_sb[:], rec[:], start=True, stop=True
                        )
                        bc_sb = recipp.tile([64, QB], fp32, tag="bcs", name="bc_sb")
                        nc.vector.tensor_copy(bc_sb[:], bc[:])
                        nc.vector.tensor_mul(
                            ocT[h * 64 : (h + 1) * 64, col0 : col0 + QB],
                            po[0:64, :],
                            bc_sb[:],
                        )

            # ---- P3: AllGather attention outputs across cores ----
            ag_in = dramp.tile([128, NQF], fp32r, name="ag_in")
            ag_out = dramp.tile(
                [NCORES * 128, NQF], fp32r, addr_space="Shared", name="ag_out"
            )
            nc.gpsimd.dma_start(ag_in[:], ocT[:])
            nc.gpsimd.collective_compute(
                "AllGather",
                mybir.AluOpType.bypass,
                replica_groups=[list(range(NCORES))],
                ins=[ag_in.opt()],
                outs=[ag_out.opt()],
            )

            # ---- P4: column-sharded output projection + bias ----
            for blk in range(NQF // QB):
                psx = psA.tile([128, QB], fp32, tag="psA", name="psx")
                for jc in range(DCH):
                    ot = iostream.tile([128, QB], fp32r, tag="ios", name="ot")
                    nc.sync.dma_start(
                        ot[:],
                        ag_out[jc * 128 : (jc + 1) * 128, blk * QB : (blk + 1) * QB],
                    )
                    nc.tensor.matmul(
                        psx[:],
                        wp_sb[:, jc, :],
                        ot[:],
                        start=(jc == 0),
                        stop=(jc == DCH - 1),
                    )
                xt = xpool.tile([128, QB], fp32, tag="xt", name="xt")
                nc.vector.tensor_scalar_add(xt[:], psx[:], bp_sb[:])
                nc.sync.dma_start(xout_d[:, blk * QB : (blk + 1) * QB], xt[:])

    nc.finalize()
    return nc


def _prep_inputs(q, k, v, key_padding_mask, Wq, Wk, Wv, Wp, bp):
    f32 = np.float32
    bf16 = ml_dtypes.bfloat16
    q = np.asarray(q, f32).reshape(NQF, D)
    k = np.asarray(k, f32).reshape(NKF, D)
    v = np.asarray(v, f32).reshape(NKF, D)
    qT = np.ascontiguousarray(q.T)
    kT = np.ascontiguousarray(k.T)
    vT = np.ascontiguousarray(v.T).astype(bf16)
    mask = np.asarray(key_padding_mask).astype(bool).reshape(B, NK)
    m01 = np.where(mask, 0.0, 1.0).astype(f32)
    # m01_sb[p, g] with flat k-row index 128*g + p
    m01_sb = np.ascontiguousarray(m01.reshape(NKF).reshape(NKC, 128).T)  # [128, 64]
    Wq = np.asarray(Wq, f32)
    Wk = np.asarray(Wk, f32)
    Wv = np.asarray(Wv, f32)
    Wp = np.asarray(Wp, f32)
    bp = np.asarray(bp, f32)
    in_maps = []
    for c in range(NCORES):
        sl = slice(c * DC, (c + 1) * DC)
        in_maps.append(
            {
                "qT": qT,
                "kT": kT,
                "vT": vT,
                "wq": np.ascontiguousarray(Wq[sl, :].T),
                "wk": np.ascontiguousarray(Wk[sl, :].T),
                "wv": np.ascontiguousarray(Wv[sl, :].T).astype(bf16),
                "wp": np.ascontiguousarray(Wp[sl, :].T),
                "bp": np.ascontiguousarray(bp[sl].reshape(DC, 1)),
                "m01": m01_sb,
            }
        )
    return in_maps


def run(trace=False, **inputs):
    """Build (cached), run on 8 cores, return (output, BassKernelResults)."""
    from concourse.bass_utils import run_bass_kernel_spmd

    if "nc" not in _CACHE:
        _CACHE["nc"] = _build_bass()
    nc = _CACHE["nc"]
    in_maps = _prep_inputs(**inputs)
    res = run_bass_kernel_spmd(nc, in_maps, list(range(NCORES)), trace=trace)
    xT = np.concatenate(
        [res.results[c]["xout"] for c in range(NCORES)], axis=0
    )  # [D, NQF]
    out = np.ascontiguousarray(xT.T).reshape(B, NQ, D).astype(np.float32)
    return out, res


def kernel(**inputs):
    out, _ = run(trace=False, **inputs)
    return out
